# revision 32
# baseline (speedup 1.0000x reference)
"""Trainium2 Bass kernel for BinaryPositionEmbedding.

out[i] = sum over set bits b of x_flat[i] of embedding[b]
       = bits[i, :13] @ embedding[:13]           (bits in {0,1})

Strategy (data-parallel over 8 NeuronCores, 4096 rows each; the 128 MiB
f32 output write is the roofline at ~358 GB/s per core ≈ 47 us):
  - Host: scale embedding[b] by the exact power of two 2^-b, split into
    bf16 hi + lo parts ([13, 1024] each). The bit-matrix rows are masked
    values (x & 2^b) in {0, 2^b} — exact in bf16 — and the hi and lo
    products accumulate into the same f32 PSUM tile (two K=13 matmuls,
    start/stop), reproducing the f32 product to ~2e-6 Frobenius rel err
    while the bits occupy only 13 partitions.
  - Device, per core: x arrives UNREPLICATED [1, 4096] int16 (8 KiB);
    GpSimd partition_broadcast fans it out to the 13 bit-partitions in
    256-col blocks, off the DMA critical path. Masked bits via DVE
    tensor_tensor bitwise_and against per-partition masks (free-dim
    broadcast), int16 -> bf16 cast on GpSimd; per 128-row chunk: 2x2
    accumulated matmuls (N=512, K=13) into PSUM, PSUM->SBUF copies on
    ScalarE (ACT is faster from PSUM and leaves DVE free), one
    contiguous 512 KB store per chunk on the SP HWDGE ring.
  - Steady-state (repeated invocation): the embedding halves and bit
    masks are loop-invariant weights, loaded ONCE outside the rep loop
    (weight-stationary, on the SP ring which is empty pre-store); the
    per-rep x load rides the ACT HWDGE ring so it never queues behind
    the 32 output stores on the SP ring (rings are FIFO — that cost the
    old kernel a ~5 us inter-rep bubble); all tile pools persist across
    reps so buffer rotation continues seamlessly at the rep boundary
    (per-rep pools made the next rep's first PSUM->SBUF copy alias the
    previous rep's LAST stage buffer, serializing on its store). With 8
    stage buffers and 2x-buffered x/bits, the next rep's compute runs
    ~8 chunks ahead under the previous rep's store stream, keeping the
    store DMAs back-to-back at the ~360 GB/s HBM-per-core limit: 32 x
    512 KiB / 360 GB/s = 46.6 us steady-state per rep (cost-model
    floor; was 52.5 us).
"""

import numpy as np
import ml_dtypes

import concourse.bass as bass
import concourse.mybir as mybir
import concourse.tile as tile
from concourse import bacc
from concourse.bass_utils import run_bass_kernel_spmd

N_CORES = 8
P = 128
D_MODEL = 1024
N_BITS = 13
K = 2 * N_BITS  # hi + lo stacked in the emb table
KB = N_BITS     # bits partitions; hi/lo share them via PSUM accumulation
N_TOTAL = 32768
ROWS = N_TOTAL // N_CORES  # 4096 rows per core
NSPLIT = 2  # matmul N tiles of 512
X_BROADCAST = True  # module default for the [1, rows] x + GpSimd fan-out


def load_weights(tc, wpool, emb_ap, sh_ap, loads_on_act=False):
    """Load the loop-invariant weights (scaled hi/lo embedding halves and
    per-partition bit masks) into persistent SBUF tiles. Both halves start
    at partition 0 so each serves as a K=13 matmul rhs. These one-time
    loads default to the SP ring: it is empty before the first store, so
    they don't delay the per-rep x loads on the ACT ring."""
    nc = tc.nc
    ldma = nc.scalar if loads_on_act else nc.sync
    emb_hi = wpool.tile([KB, D_MODEL], mybir.dt.bfloat16)
    emb_lo = wpool.tile([KB, D_MODEL], mybir.dt.bfloat16)
    sh_t = wpool.tile([KB, 1], mybir.dt.int16)
    ldma.dma_start(sh_t[:], sh_ap)
    ldma.dma_start(emb_hi[:], emb_ap[:KB])
    ldma.dma_start(emb_lo[:], emb_ap[KB:])
    return (emb_hi, emb_lo), sh_t


def build_body(
    tc,
    out_ap,
    x_ap,
    emb_t,
    sh_t,
    pools,            # (xpool, spool, ppool) shared across reps so buffer
                      # rotation continues seamlessly at the rep boundary
    rows,
    dma_batch=1,      # chunks per output dma_start
    act_every=1,      # of every act_every copies, 1 goes to ScalarE
    bits_block=256,   # columns per bits-pipeline step (also x DMA split)
    bits_direct=False,  # single AND writing bf16 directly (walrus rejects)
    mix_early=0,      # chunks at the start whose copies alternate ACT/DVE
    half_chunks=0,    # chunks at the start DMAed per 512-col half
    bits_engine="vector",  # "vector" (DVE); "pool" can't int-op (walrus)
    loads_on_act=True,  # input loads on the ACT HWDGE ring, not the store ring
    x_broadcast=None,  # x arrives [1, rows]; replicate on GpSimd, not DMA
    bcast_block=256,  # columns per partition_broadcast call
    x_on_swdge=False,  # x loads via GpSimd SWDGE (no HWDGE desc-gen queue)
):
    """Emit the per-core, per-rep program. out_ap [rows, 1024] f32;
    x_ap [13, rows] i16 (x replicated across partitions); emb_t = (hi, lo)
    [13, 1024] bf16 tiles (parts of embedding[b] * 2^-b); sh_t [13, 1] i16
    tile = 1 << b per-partition bit masks. bits become 0 or 2^b, exact in
    bf16; the 2^-b scaling folded into emb keeps the product exact; the
    hi and lo products accumulate in the same f32 PSUM tile, so K=13 bits
    serve both halves."""
    nc = tc.nc
    chunks = rows // P
    out_v = out_ap.rearrange("(m c p) d -> m p c d", c=dma_batch, p=P)
    xpool, spool, ppool = pools
    emb_hi, emb_lo = emb_t

    if True:
        bits_block = min(bits_block, rows)
        x_t = xpool.tile([KB, rows], mybir.dt.int16, name="x_t")
        # two-piece x load: a small head so the first bits block starts
        # early, then the remainder in one large transfer. Loads ride the
        # ACT HWDGE ring: the store ring is FIFO, so a load queued there
        # would wait behind the previous iteration's 32 stores (~5 us
        # inter-iteration bubble).
        ldma = nc.scalar if loads_on_act else nc.sync
        if x_broadcast is None:
            x_broadcast = X_BROADCAST
        xone_t = None
        if x_broadcast:
            # x arrives unreplicated [1, rows] (8 KiB, ~24 ns of DMA);
            # GpSimd fans it out to the 13 bit-partitions off the DMA
            # critical path. The broadcasts are emitted inside emit_bits,
            # interleaved with the casts, so the first matmul doesn't wait
            # for the full-row fan-out.
            xone_t = xpool.tile([1, rows], mybir.dt.int16, name="xone_t")
            xdma = nc.gpsimd if x_on_swdge else ldma
            xdma.dma_start(xone_t[:, :bits_block], x_ap[:, :bits_block])
            if rows > bits_block:
                xdma.dma_start(xone_t[:, bits_block:], x_ap[:, bits_block:])
        else:
            ldma.dma_start(x_t[:, :bits_block], x_ap[:, :bits_block])
            if rows > bits_block:
                ldma.dma_start(x_t[:, bits_block:], x_ap[:, bits_block:])

        bits_i = (
            None
            if bits_direct
            else xpool.tile([KB, rows], mybir.dt.int16, name="bits_i")
        )
        bits_t = xpool.tile([KB, rows], mybir.dt.bfloat16, name="bits_t")
        beng = nc.vector if bits_engine == "vector" else nc.gpsimd

        def emit_bits(q):
            sl = slice(q * bits_block, (q + 1) * bits_block)
            if x_broadcast and (q * bits_block) % bcast_block == 0:
                bsl = slice(
                    q * bits_block, min(q * bits_block + bcast_block, rows)
                )
                nc.gpsimd.partition_broadcast(
                    x_t[:, bsl], xone_t[:, bsl], channels=KB
                )
            if bits_direct:
                beng.tensor_tensor(
                    bits_t[:, sl],
                    x_t[:, sl],
                    sh_t[:].to_broadcast((KB, bits_block)),
                    mybir.AluOpType.bitwise_and,
                )
            else:
                beng.tensor_tensor(
                    bits_i[:, sl],
                    x_t[:, sl],
                    sh_t[:].to_broadcast((KB, bits_block)),
                    mybir.AluOpType.bitwise_and,
                )
                nc.gpsimd.tensor_copy(bits_t[:, sl], bits_i[:, sl])

        def emit_chunk_group(m, head, half=False):
            stg = spool.tile(
                [P, dma_batch, D_MODEL], mybir.dt.float32, name="stg"
            )
            for c in range(dma_batch):
                n = m * dma_batch + c
                lhsT = bits_t[:, n * P : (n + 1) * P]
                for j in range(NSPLIT):
                    nsl = slice(j * 512, (j + 1) * 512)
                    ps = ppool.tile([P, 512], mybir.dt.float32, name="ps")
                    nc.tensor.matmul(
                        ps[:], lhsT, emb_hi[:, nsl], start=True, stop=False
                    )
                    nc.tensor.matmul(
                        ps[:], lhsT, emb_lo[:, nsl], start=False, stop=True
                    )
                    if head:
                        use_act = j % 2 == 0  # parallel ACT+DVE staging
                    else:
                        use_act = emit_chunk_group.copy_idx % act_every == 0
                    if use_act:
                        nc.scalar.copy(stg[:, c, nsl], ps[:])
                    else:
                        nc.vector.tensor_copy(stg[:, c, nsl], ps[:])
                    emit_chunk_group.copy_idx += 1
                    if half:
                        nc.sync.dma_start(out_v[m, :, c, nsl], stg[:, c, nsl])
            if not half:
                # head chunks ride the otherwise-empty ACT HWDGE ring
                (nc.scalar if head else nc.sync).dma_start(out_v[m], stg[:])

        emit_chunk_group.copy_idx = 0
        n_blocks = rows // bits_block
        head_groups = min(mix_early, chunks // dma_batch)
        head_blocks = min(
            n_blocks, (head_groups * dma_batch * P + bits_block - 1) // bits_block
        )
        # ramp: first bits block(s), then the head chunks with parallel
        # ACT/DVE staging, then the remaining bits, then the bulk
        for q in range(head_blocks):
            emit_bits(q)
        for m in range(head_groups):
            emit_chunk_group(m, head=True)
        for q in range(head_blocks, n_blocks):
            emit_bits(q)
        for m in range(head_groups, chunks // dma_batch):
            emit_chunk_group(m, head=False, half=m < half_chunks)


def _build_nc(
    rows=ROWS,
    reps=1,
    weight_kwargs=None,
    x_bufs=2,
    stage_bufs=8,
    psum_bufs=8,
    **body_kwargs,
):
    nc = bacc.Bacc(
        "TRN2", target_bir_lowering=False, debug=False, enable_asserts=False
    )
    xp = 1 if body_kwargs.get("x_broadcast", X_BROADCAST) else KB
    x_in = nc.dram_tensor("xrep", [xp, rows], mybir.dt.int16, kind="ExternalInput")
    emb_in = nc.dram_tensor(
        "embhl", [K, D_MODEL], mybir.dt.bfloat16, kind="ExternalInput"
    )
    sh_in = nc.dram_tensor("shifts", [KB, 1], mybir.dt.int16, kind="ExternalInput")
    out = nc.dram_tensor(
        "out", [rows, D_MODEL], mybir.dt.float32, kind="ExternalOutput"
    )
    wkw = dict(weight_kwargs or {})
    if "weights_on_act" in body_kwargs:
        wkw["loads_on_act"] = body_kwargs.pop("weights_on_act")
    with tile.TileContext(nc) as tc:
        with (
            tc.tile_pool(name="wpool", bufs=1) as wpool,
            tc.tile_pool(name="xpool", bufs=x_bufs) as xpool,
            tc.tile_pool(name="stage", bufs=stage_bufs) as spool,
            tc.tile_pool(name="psum", bufs=psum_bufs, space="PSUM") as ppool,
        ):
            emb_t, sh_t = load_weights(tc, wpool, emb_in.ap(), sh_in.ap(), **wkw)
            pools = (xpool, spool, ppool)
            if reps == 1:
                build_body(
                    tc, out.ap(), x_in.ap(), emb_t, sh_t, pools, rows,
                    **body_kwargs,
                )
            else:
                with tc.For_i(0, reps, 1):
                    build_body(
                        tc, out.ap(), x_in.ap(), emb_t, sh_t, pools, rows,
                        **body_kwargs,
                    )
    nc.finalize()
    return nc


_NC_CACHE = {}


def make_in_maps(x, embedding, x_broadcast=None):
    if x_broadcast is None:
        x_broadcast = X_BROADCAST
    xp = 1 if x_broadcast else KB
    x_flat = np.ascontiguousarray(np.asarray(x).reshape(-1).astype(np.int16))
    emb13 = np.asarray(embedding)[:N_BITS].astype(np.float32)
    # bits arrive as 0 or 2^b; fold the exact 2^-b scale into the table
    scaled = emb13 * (0.5 ** np.arange(N_BITS, dtype=np.float32))[:, None]
    hi = scaled.astype(ml_dtypes.bfloat16)
    lo = (scaled - hi.astype(np.float32)).astype(ml_dtypes.bfloat16)
    embhl = np.ascontiguousarray(np.concatenate([hi, lo], axis=0))
    shifts = (1 << np.arange(KB, dtype=np.int32)).astype(np.int16).reshape(KB, 1)
    in_maps = []
    for c in range(N_CORES):
        shard = x_flat[c * ROWS : (c + 1) * ROWS]
        in_maps.append(
            {
                "xrep": np.ascontiguousarray(
                    np.broadcast_to(shard, (xp, ROWS))
                ),
                "embhl": embhl,
                "shifts": shifts,
            }
        )
    return in_maps


def kernel(x, embedding, **run_kwargs):
    if "nc" not in _NC_CACHE:
        _NC_CACHE["nc"] = _build_nc()
    nc = _NC_CACHE["nc"]
    in_maps = make_in_maps(x, embedding)
    res = run_bass_kernel_spmd(
        nc, in_maps, core_ids=list(range(N_CORES)), **run_kwargs
    )
    out = np.concatenate([r["out"] for r in res.results], axis=0)
    if run_kwargs:
        kernel.last_results = res
    return out
